# revision 7
# baseline (speedup 1.0000x reference)
"""Trainium2 Bass kernel: Convpass adapter with hypernet-generated 3x3 conv.

Per core (data-parallel over batch, 8 images/core):
  hypernet: conv_w = emb @ w_hyper + b_hyper. Sharded 8 ways over the conv
            output-channel index i: each core computes its 8 i-rows from a
            host-packed 0.59MB slice (diag-window matmul trick, both
            o-halves packed on 128 partitions -> 8 matmuls of N=288), then
            an HBM AllGather shares the 9KB blocks.
  down:     xT[128c,4k,784] @ [w_down|w_down] -> psum [128, 392] per half
            (x arrives pre-transposed bf16 from the host; psum rows 0-63 ==
            rows 64-127 so the conv can pack 2 taps)
  gelu1:    quickgelu(. + b_down), written twice: rows 0-63 at col+1 (dx=0
            taps), rows 64-127 at col (dx=1 taps) of a padded buffer
  conv:     3x3 as 3 K=128 matmuls (dx=0,1 packed) + 3 K=64 (dx=2)
  gelu2:    quickgelu(. * scale)
  up:       out^T[128c,392] = w_up65[:,cslice].T @ y_act  (stationary w_up,
            ones-row fused bias); stored transposed bf16, host untransposes.

The image loop is software-pipelined for the in-order PE queue: tensor
order is conv(i), down(i+1), up(i) so vector/scalar gelu latencies hide
under matmuls. The first image's down/gelu1 and its x_act memsets are
issued BEFORE the collective so no per-image work queues behind it.
All matmul inputs are bf16; accumulation is fp32 in PSUM.
"""

import os

import numpy as np
import ml_dtypes

import concourse.bass as bass
import concourse.mybir as mybir
import concourse.tile as tile
from concourse import bacc
from concourse.bass_utils import run_bass_kernel_spmd

# Problem shapes (hardcoded per contract).
B, H, W, C = 64, 28, 28, 512
DIM, EMB = 64, 64
NCORES = 8
B_LOC = B // NCORES            # 8 images per core
PIX = H * W                    # 784 pixels per image
PW = W + 2                     # 30 padded width
PAD = PW * (H + 2)             # 900 padded pixels per image
RH = 2                         # row-halves per image
RROWS = H // RH                # 14 rows per half
NHALF = RROWS * W              # 392 pixels per half-tile
KCH = C // 128                 # 4 contraction chunks of 128 channels
JTOT = DIM * DIM * 9           # 36864 hypernet outputs
NHYP = 32 * 9                  # 288 = free size of packed hypernet matmuls
ISH = DIM // NCORES            # 8 i-rows of conv_w per core

F32 = mybir.dt.float32
BF16 = mybir.dt.bfloat16
GELU_A = 1.702

# Debug escape hatch: set KERNEL_DEBUG_NO_CC=1 to build without the
# AllGather (each core computes the full hypernet) e.g. for CoreSim.
USE_CC = os.environ.get("KERNEL_DEBUG_NO_CC", "") != "1"

_CACHE = {}


def build_kernel():
    key = ("cc" if USE_CC else "nocc")
    if key in _CACHE:
        return _CACHE[key]

    nc = bacc.Bacc("TRN2", target_bir_lowering=False, debug=False)

    n_i = ISH if USE_CC else DIM
    x_d = nc.dram_tensor("x", [B_LOC, 128, KCH * PIX], BF16, kind="ExternalInput")
    wd_d = nc.dram_tensor("w_down", [C, DIM], F32, kind="ExternalInput")
    bd_d = nc.dram_tensor("b_down", [DIM], F32, kind="ExternalInput")
    wu_d = nc.dram_tensor("w_up", [DIM, C], F32, kind="ExternalInput")
    bu_d = nc.dram_tensor("b_up", [C], F32, kind="ExternalInput")
    sc_d = nc.dram_tensor("scale", [DIM], F32, kind="ExternalInput")
    # host-packed hypernet slice: [128, il, ol, t] with rows 0-63 = o<32,
    # rows 64-127 = o>=32 (see _make_in_maps)
    wh_d = nc.dram_tensor("w_hyper", [128, n_i * NHYP], BF16, kind="ExternalInput")
    # host-packed bias: [i, (o t)] bf16
    bh_d = nc.dram_tensor("b_hyper", [DIM, DIM * 9], BF16, kind="ExternalInput")
    emb_d = nc.dram_tensor("layer_emb", [EMB], F32, kind="ExternalInput")
    out_d = nc.dram_tensor("out", [B_LOC, 128, KCH * PIX], BF16, kind="ExternalOutput")

    with tile.TileContext(nc) as tc:
        with tc.tile_pool(name="consts", bufs=1) as consts:
            # ---- constants / small params ----
            # w_down duplicated along M so the down matmul writes identical
            # psum halves: [128c, k, 128m] with m 0-63 == m 64-127.
            w_down2 = consts.tile([128, KCH, 128], BF16)
            for half in range(2):
                nc.gpsimd.dma_start(
                    w_down2[:, :, half * DIM : (half + 1) * DIM],
                    wd_d[:].rearrange("(k p) d -> p k d", p=128),
                )
            w_up65 = consts.tile([DIM + 1, C], BF16)
            nc.gpsimd.dma_start(w_up65[:DIM, :], wu_d[:])
            nc.gpsimd.dma_start(w_up65[DIM : DIM + 1, :], bu_d[:][None, :])

            b_down2 = consts.tile([128, 1], F32)
            for half in range(2):
                nc.sync.dma_start(
                    b_down2[half * DIM : (half + 1) * DIM, :], bd_d[:][:, None]
                )
            b_down_g2 = consts.tile([128, 1], F32)
            nc.vector.tensor_scalar_mul(b_down_g2[:], b_down2[:], GELU_A)

            scale_sb = consts.tile([DIM, 1], F32)
            nc.sync.dma_start(scale_sb[:], sc_d[:][:, None])
            scale_g = consts.tile([DIM, 1], F32)
            nc.vector.tensor_scalar_mul(scale_g[:], scale_sb[:], GELU_A)

            # hypernet lhsT: zeros except column 64 = [emb;0], col 128 =
            # [0;emb]; window t2[:, 64-i : 192-i] puts emb into output
            # partitions i (o<32 rows) and 64+i (o>=32 rows).
            t2 = consts.tile([128, 192], BF16)
            nc.gpsimd.memset(t2[:], 0.0)
            nc.gpsimd.dma_start(t2[0:EMB, 64:65], emb_d[:][:, None])
            nc.gpsimd.dma_start(t2[EMB:128, 128:129], emb_d[:][:, None])

            if USE_CC:
                b_prep = consts.tile([DIM, DIM * 9], BF16)
                nc.gpsimd.dma_start(b_prep[:], bh_d[:])
            else:
                # split layout matching the packed psum rows: rows 0-63 =
                # bias[i, o<32], rows 64-127 = bias[i, o>=32]
                b_prep2 = consts.tile([128, NHYP], BF16)
                nc.gpsimd.dma_start(b_prep2[0:DIM, :], bh_d[:, 0:NHYP])
                nc.gpsimd.dma_start(b_prep2[DIM:, :], bh_d[:, NHYP:])

            # rows 0-63: W[i, o*9+t]; rows 64-127: same shifted by one tap so
            # a K=128 conv matmul contracts taps (dy,0) and (dy,1) at once.
            w_conv2 = consts.tile([128, DIM * 9], BF16)

            # ---- main pools ----
            with (
                tc.tile_pool(name="whpool", bufs=1) as whpool,
                tc.tile_pool(name="dram", bufs=1, space="DRAM") as dram,
                tc.tile_pool(name="xin", bufs=3) as xin,
                tc.tile_pool(name="xact", bufs=3) as xactp,
                tc.tile_pool(name="yact", bufs=3) as yactp,
                tc.tile_pool(name="tmp", bufs=6) as tmpp,
                tc.tile_pool(name="outs", bufs=2) as outsp,
                tc.tile_pool(name="ps_s", bufs=4, space="PSUM") as ps_sp,
                tc.tile_pool(name="ps_c", bufs=2, space="PSUM") as ps_cp,
                tc.tile_pool(name="ps_u", bufs=2, space="PSUM") as ps_up,
            ):
                # ---- prologue ----
                wh_sb = whpool.tile([128, n_i * NHYP], BF16, tag="wh")
                nc.scalar.dma_start(wh_sb[:], wh_d[:])

                def load_x(img):
                    xT = xin.tile([128, KCH, PIX], BF16, tag="x", name=f"x{img}")
                    nc.sync.dma_start(
                        xT[:].rearrange("p k n -> p (k n)"), x_d[img]
                    )
                    return xT

                xTs = [load_x(0), load_x(1)]

                def make_xact(img):
                    x_act = xactp.tile([128, PAD], BF16, tag="xa", name=f"xa{img}")
                    nc.gpsimd.memset(x_act[:], 0.0)
                    return x_act

                xacts = [make_xact(0), make_xact(1)]

                # hypernet matmuls: psum row il collects W[i0+il, o<32],
                # row 64+il collects W[i0+il, o>=32] (disjoint rows, one
                # accumulation group).
                wh_v = wh_sb[:].rearrange("p (il f) -> p il f", f=NHYP)
                ps_w = ps_up.tile([128, NHYP], F32, tag="psu", name="hyp")
                for il in range(n_i):
                    nc.tensor.matmul(
                        ps_w[:],
                        t2[:, 64 - il : 192 - il],
                        wh_v[:, il, :],
                        start=(il == 0),
                        stop=(il == n_i - 1),
                    )

                def down(img, xT):
                    """down-proj matmuls -> 2 psum tiles [128, 392]"""
                    ps_ds = [
                        ps_sp.tile([128, NHALF], F32, tag="pss", name=f"psd{img}_{rh}")
                        for rh in range(RH)
                    ]
                    for k in range(KCH):
                        for rh in range(RH):
                            nc.tensor.matmul(
                                ps_ds[rh][:],
                                w_down2[:, k, :],
                                xT[:, k, rh * NHALF : (rh + 1) * NHALF],
                                start=(k == 0),
                                stop=(k == KCH - 1),
                            )
                    return ps_ds

                def gelu1(img, ps_ds, x_act):
                    """quickgelu -> padded interior of x_act (both copies)"""
                    x_act_v = x_act[:].rearrange("d (r c) -> d r c", c=PW)
                    for rh in range(RH):
                        ps_d = ps_ds[rh]
                        t_t = tmpp.tile([128, NHALF], BF16, tag="t")
                        nc.vector.tensor_scalar_add(t_t[:], ps_d[:], b_down2[:])
                        s_t = tmpp.tile([128, NHALF], BF16, tag="s")
                        nc.scalar.activation(
                            s_t[:],
                            ps_d[:],
                            mybir.ActivationFunctionType.Sigmoid,
                            bias=b_down_g2[:],
                            scale=GELU_A,
                        )
                        rows = slice(1 + rh * RROWS, 1 + (rh + 1) * RROWS)
                        nc.vector.tensor_tensor(
                            x_act_v[:DIM, rows, 1 : 1 + W],
                            t_t[:DIM].rearrange("d (r c) -> d r c", c=W),
                            s_t[:DIM].rearrange("d (r c) -> d r c", c=W),
                            mybir.AluOpType.mult,
                        )
                        nc.vector.tensor_tensor(
                            x_act_v[DIM:, rows, 0:W],
                            t_t[DIM:].rearrange("d (r c) -> d r c", c=W),
                            s_t[DIM:].rearrange("d (r c) -> d r c", c=W),
                            mybir.AluOpType.mult,
                        )
                    return x_act_v

                # first image's down + gelu1 issued before the collective so
                # nothing per-image waits behind it in any queue
                ps_cur = down(0, xTs[0])
                xact_cur = gelu1(0, ps_cur, xacts[0])

                if USE_CC:
                    blk = whpool.tile([128, NHYP], BF16, tag="blk")
                    nc.vector.tensor_copy(blk[:], ps_w[:])
                    cc_in = dram.tile([2 * ISH, NHYP], BF16)
                    nc.gpsimd.dma_start(cc_in[0:ISH, :], blk[0:ISH, :])
                    nc.gpsimd.dma_start(cc_in[ISH:, :], blk[DIM : DIM + ISH, :])
                    cc_out = dram.tile([2 * DIM, NHYP], BF16)
                    nc.gpsimd.collective_compute(
                        "AllGather",
                        mybir.AluOpType.bypass,
                        replica_groups=[list(range(NCORES))],
                        ins=[cc_in[:].opt()],
                        outs=[cc_out[:].opt()],
                    )
                    # cc_out rows = [(core, half, il)] -> w_conv2[i=8c+il]
                    wtmp = whpool.tile([DIM, DIM * 9], BF16, tag="wtmp")
                    cc_v = cc_out[:].rearrange(
                        "(c hh il) f -> c hh il f", hh=2, il=ISH
                    )
                    for hh in range(2):
                        nc.gpsimd.dma_start(
                            wtmp[:, hh * NHYP : (hh + 1) * NHYP].rearrange(
                                "(c il) f -> c il f", il=ISH
                            ),
                            cc_v[:, hh],
                        )
                    nc.vector.tensor_tensor(
                        w_conv2[:DIM, :], wtmp[:], b_prep[:], mybir.AluOpType.add
                    )
                else:
                    # full hypernet on every core: psum rows 0-63 = o<32
                    # block, rows 64-127 = o>=32 block
                    wtmp = whpool.tile([128, NHYP], BF16, tag="wtmp")
                    nc.vector.tensor_tensor(
                        wtmp[:], ps_w[:], b_prep2[:], mybir.AluOpType.add
                    )
                    nc.vector.tensor_copy(w_conv2[:DIM, :NHYP], wtmp[:DIM, :])
                    nc.gpsimd.dma_start(w_conv2[:DIM, NHYP:], wtmp[DIM:, :])
                # bottom half = top shifted by one tap (partition move -> DMA)
                nc.gpsimd.dma_start(
                    w_conv2[DIM:, : DIM * 9 - 1], w_conv2[:DIM, 1 : DIM * 9]
                )
                nc.vector.memset(w_conv2[DIM:, DIM * 9 - 1 :], 0.0)
                w_conv_v = w_conv2[:].rearrange("i (o t) -> i o t", t=9)

                for img in range(B_LOC):
                    # conv: dx=0,1 packed (K=128) + dx=2 (K=64)
                    ps_cs = []
                    for rh in range(RH):
                        ps_c = ps_cp.tile(
                            [DIM, NHALF], F32, tag="psc", name=f"psc{img}_{rh}"
                        )
                        first = True
                        for dy in range(3):
                            src = xact_cur[
                                :, rh * RROWS + dy : rh * RROWS + dy + RROWS, 0:W
                            ]
                            nc.tensor.matmul(
                                ps_c[:],
                                w_conv_v[:, :, dy * 3],
                                src,
                                start=first,
                                stop=False,
                            )
                            first = False
                        for dy in range(3):
                            src = xact_cur[
                                :DIM,
                                rh * RROWS + dy : rh * RROWS + dy + RROWS,
                                2 : 2 + W,
                            ]
                            nc.tensor.matmul(
                                ps_c[:],
                                w_conv_v[:DIM, :, dy * 3 + 2],
                                src,
                                start=False,
                                stop=(dy == 2),
                            )
                        ps_cs.append(ps_c)

                    # pipelined: issue next image's load+down (tensor queue
                    # stays busy while gelu2 below runs on vector/scalar)
                    if img + 1 < B_LOC:
                        if img + 2 < B_LOC:
                            xTs.append(load_x(img + 2))
                            xacts.append(make_xact(img + 2))
                        ps_nxt = down(img + 1, xTs[img + 1])
                    else:
                        ps_nxt = None

                    # gelu2 -> y_act (ones row fuses the up bias)
                    y_act = yactp.tile([DIM + 1, PIX], BF16, tag="ya")
                    nc.gpsimd.memset(y_act[DIM : DIM + 1, :], 1.0)
                    for rh in range(RH):
                        ps_c = ps_cs[rh]
                        t2s = tmpp.tile([DIM, NHALF], BF16, tag="t")
                        nc.vector.tensor_scalar_mul(t2s[:], ps_c[:], scale_sb[:])
                        s2 = tmpp.tile([DIM, NHALF], BF16, tag="s")
                        nc.scalar.activation(
                            s2[:],
                            ps_c[:],
                            mybir.ActivationFunctionType.Sigmoid,
                            bias=0.0,
                            scale=scale_g[:],
                        )
                        nc.vector.tensor_tensor(
                            y_act[:DIM, rh * NHALF : (rh + 1) * NHALF],
                            t2s[:],
                            s2[:],
                            mybir.AluOpType.mult,
                        )

                    # up-proj + bias, transposed: out^T[c,pix] per c-chunk
                    o_sb = outsp.tile([128, KCH, PIX], BF16, tag="o")
                    for kc in range(KCH):
                        for rh in range(RH):
                            ps_u = ps_up.tile([128, NHALF], F32, tag="psu")
                            nc.tensor.matmul(
                                ps_u[:],
                                w_up65[:, kc * 128 : (kc + 1) * 128],
                                y_act[:, rh * NHALF : (rh + 1) * NHALF],
                                start=True,
                                stop=True,
                            )
                            dst = o_sb[:, kc, rh * NHALF : (rh + 1) * NHALF]
                            if (kc * RH + rh) % 2 == 0:
                                nc.scalar.copy(dst, ps_u[:])
                            else:
                                nc.vector.tensor_copy(dst, ps_u[:])
                    nc.scalar.dma_start(
                        out_d[img], o_sb[:].rearrange("p k n -> p (k n)")
                    )

                    if ps_nxt is not None:
                        xact_cur = gelu1(img + 1, ps_nxt, xacts[img + 1])
                        ps_cur = ps_nxt

    nc.compile()
    _CACHE[key] = nc
    return nc


def _pack_hyper_full(w_hyper_bf16):
    """[64e, 36864] -> [128, n_i, 32, 9] packed: rows 0-63 stream the o<32
    block, rows 64-127 the o>=32 block; free layout [il, ol, t]."""
    wh = np.asarray(w_hyper_bf16).reshape(EMB, DIM, DIM, 9)  # [e, o, i, t]
    top = wh[:, :32].transpose(0, 2, 1, 3)  # [e, i, ol, t]
    bot = wh[:, 32:].transpose(0, 2, 1, 3)
    return np.concatenate([top, bot], axis=0)  # [128, i, ol, t]


def _make_in_maps(inputs):
    bf16 = ml_dtypes.bfloat16
    x = np.ascontiguousarray(inputs["x"], dtype=np.float32)
    shared = {
        k: np.ascontiguousarray(inputs[k], np.float32)
        for k in ("w_down", "b_down", "w_up", "b_up", "scale", "layer_emb")
    }
    # bias pre-arranged to [i, (o t)] bf16
    bh = np.asarray(inputs["b_hyper"], np.float32).reshape(DIM, DIM, 9)
    shared["b_hyper"] = np.ascontiguousarray(
        bh.transpose(1, 0, 2).reshape(DIM, DIM * 9)
    ).astype(bf16)

    whb = np.asarray(inputs["w_hyper"], np.float32).astype(bf16)
    packed = _pack_hyper_full(whb)  # [128, i, ol, t]
    if USE_CC:
        wh_packs = [
            np.ascontiguousarray(
                packed[:, c * ISH : (c + 1) * ISH].reshape(128, ISH * NHYP)
            )
            for c in range(NCORES)
        ]
    else:
        full = np.ascontiguousarray(packed.reshape(128, DIM * NHYP))
        wh_packs = [full] * NCORES

    in_maps = []
    for c in range(NCORES):
        xc = x[c * B_LOC : (c + 1) * B_LOC].reshape(B_LOC, PIX, KCH, 128)
        xt = np.ascontiguousarray(xc.transpose(0, 3, 2, 1)).astype(bf16)
        in_maps.append(
            {"x": xt.reshape(B_LOC, 128, KCH * PIX), "w_hyper": wh_packs[c], **shared}
        )
    return in_maps


def _untranspose_out(res):
    outs = []
    for c in range(NCORES):
        o = np.asarray(res.results[c]["out"]).reshape(B_LOC, 128, KCH, PIX)
        o = o.transpose(0, 3, 2, 1).astype(np.float32)  # [img, pix, kc, p]
        outs.append(o.reshape(B_LOC, H, W, C))
    return np.concatenate(outs, axis=0)


def kernel(**inputs) -> np.ndarray:
    nc = build_kernel()
    in_maps = _make_in_maps(inputs)
    res = run_bass_kernel_spmd(nc, in_maps, core_ids=list(range(NCORES)))
    return _untranspose_out(res)


def run_traced(inputs, **kw):
    """For test.py: run with tracing to get HW exec time."""
    nc = build_kernel()
    in_maps = _make_in_maps(inputs)
    return run_bass_kernel_spmd(
        nc, in_maps, core_ids=list(range(NCORES)), trace=True, **kw
    )


# revision 14
# speedup vs baseline: 1.2554x; 1.2554x over previous
"""Trainium2 Bass kernel: Convpass adapter with hypernet-generated 3x3 conv.

Per core (data-parallel over batch, 8 images/core):
  hypernet: conv_w = emb @ w_hyper + b_hyper. Sharded 8 ways over the conv
            output-channel index i: each core computes its 8 i-rows from a
            host-packed 0.59MB slice (diag-window matmul trick, both
            o-halves packed on 128 partitions -> 8 matmuls of N=288), then
            an HBM AllGather shares the 9KB blocks.
  down:     xT[128c,4k,784] @ [w_down|w_down] -> psum [128, 392] per half
            (x arrives pre-transposed bf16 from the host; psum rows 0-63 ==
            rows 64-127 so the conv can pack 2 taps)
  gelu1:    quickgelu(. + b_down), written twice: rows 0-63 at col+1 (dx=0
            taps), rows 64-127 at col (dx=1 taps) of a padded buffer
  conv:     3x3 as 3 K=128 matmuls (dx=0,1 packed) + 3 K=64 (dx=2)
  gelu2:    quickgelu(. * scale)
  up:       out^T[128c,392] = w_up65[:,cslice].T @ y_act  (stationary w_up,
            ones-row fused bias); stored transposed bf16, host untransposes.

The image loop is software-pipelined for the in-order PE queue: tensor
order is conv(i), down(i+1), up(i) so vector/scalar gelu latencies hide
under matmuls. The first image's down/gelu1 and its x_act memsets are
issued BEFORE the collective so no per-image work queues behind it.
All matmul inputs are bf16; accumulation is fp32 in PSUM.
"""

import os

import numpy as np
import ml_dtypes

import concourse.bass as bass
import concourse.mybir as mybir
import concourse.tile as tile
from concourse import bacc
from concourse.bass_utils import run_bass_kernel_spmd

# Problem shapes (hardcoded per contract).
B, H, W, C = 64, 28, 28, 512
DIM, EMB = 64, 64
NCORES = 8
B_LOC = B // NCORES            # 8 images per core
PIX = H * W                    # 784 pixels per image
PW = W + 2                     # 30 padded width
PAD = PW * (H + 2)             # 900 padded pixels per image
RH = 2                         # row-halves per image
RROWS = H // RH                # 14 rows per half
NHALF = RROWS * W              # 392 pixels per half-tile
KCH = C // 128                 # 4 contraction chunks of 128 channels
JTOT = DIM * DIM * 9           # 36864 hypernet outputs
NHYP = 32 * 9                  # 288 = free size of packed hypernet matmuls
ISH = DIM // NCORES            # 8 i-rows of conv_w per core

F32 = mybir.dt.float32
BF16 = mybir.dt.bfloat16
GELU_A = 1.702

# The AllGather path measured a ~45us fixed collective cost on this
# platform — default to the full per-core hypernet with the w_hyper load
# chunked and interleaved into the pipeline instead.
USE_CC = os.environ.get("KERNEL_DEBUG_USE_CC", "") == "1"
NCHUNK = 8                     # w_hyper streaming DMA chunks
ICH = DIM // NCHUNK            # 8 i-rows per DMA chunk
IGRP = 32                      # i-rows per compute/build group (32-aligned)

_CACHE = {}


def build_kernel():
    key = ("cc" if USE_CC else "nocc")
    if key in _CACHE:
        return _CACHE[key]

    nc = bacc.Bacc("TRN2", target_bir_lowering=False, debug=False)

    n_i = ISH if USE_CC else DIM
    x_d = nc.dram_tensor("x", [B_LOC, 128, KCH * PIX], BF16, kind="ExternalInput")
    wd_d = nc.dram_tensor("w_down", [C, DIM], F32, kind="ExternalInput")
    bd_d = nc.dram_tensor("b_down", [DIM], F32, kind="ExternalInput")
    wu_d = nc.dram_tensor("w_up", [DIM, C], F32, kind="ExternalInput")
    bu_d = nc.dram_tensor("b_up", [C], F32, kind="ExternalInput")
    sc_d = nc.dram_tensor("scale", [DIM], F32, kind="ExternalInput")
    # host-packed hypernet slice: [128, il, ol, t] with rows 0-63 = o<32,
    # rows 64-127 = o>=32 (see _make_in_maps)
    wh_d = nc.dram_tensor("w_hyper", [128, n_i * NHYP], BF16, kind="ExternalInput")
    # host-packed bias: [i, (o t)] bf16
    bh_d = nc.dram_tensor("b_hyper", [DIM, DIM * 9], BF16, kind="ExternalInput")
    emb_d = nc.dram_tensor("layer_emb", [EMB], F32, kind="ExternalInput")
    out_d = nc.dram_tensor("out", [B_LOC, 128, KCH * PIX], BF16, kind="ExternalOutput")

    with tile.TileContext(nc) as tc:
        with tc.tile_pool(name="consts", bufs=1) as consts:
            # ---- constants / small params ----
            # w_down duplicated along M so the down matmul writes identical
            # psum halves: [128c, k, 128m] with m 0-63 == m 64-127.
            w_down2 = consts.tile([128, KCH, 128], BF16)
            for half in range(2):
                nc.gpsimd.dma_start(
                    w_down2[:, :, half * DIM : (half + 1) * DIM],
                    wd_d[:].rearrange("(k p) d -> p k d", p=128),
                )
            w_up65 = consts.tile([DIM + 1, C], BF16)
            nc.gpsimd.dma_start(w_up65[:DIM, :], wu_d[:])
            nc.gpsimd.dma_start(w_up65[DIM : DIM + 1, :], bu_d[:][None, :])

            b_down2 = consts.tile([128, 1], F32)
            for half in range(2):
                nc.sync.dma_start(
                    b_down2[half * DIM : (half + 1) * DIM, :], bd_d[:][:, None]
                )
            b_down_g2 = consts.tile([128, 1], F32)
            nc.vector.tensor_scalar_mul(b_down_g2[:], b_down2[:], GELU_A)

            scale_sb = consts.tile([DIM, 1], F32)
            nc.sync.dma_start(scale_sb[:], sc_d[:][:, None])
            scale_g = consts.tile([DIM, 1], F32)
            nc.vector.tensor_scalar_mul(scale_g[:], scale_sb[:], GELU_A)

            # hypernet lhsT: zeros except column 64 = [emb;0], col 128 =
            # [0;emb]; window t2[:, 64-i : 192-i] puts emb into output
            # partitions i (o<32 rows) and 64+i (o>=32 rows).
            t2 = consts.tile([128, 192], BF16)
            nc.gpsimd.memset(t2[:], 0.0)
            nc.gpsimd.dma_start(t2[0:EMB, 64:65], emb_d[:][:, None])
            nc.gpsimd.dma_start(t2[EMB:128, 128:129], emb_d[:][:, None])

            if USE_CC:
                b_prep = consts.tile([DIM, DIM * 9], BF16)
                nc.gpsimd.dma_start(b_prep[:], bh_d[:])
            else:
                # split layout matching the packed psum rows: rows 0-63 =
                # bias[i, o<32], rows 64-127 = bias[i, o>=32]
                b_prep2 = consts.tile([128, NHYP], BF16)
                nc.gpsimd.dma_start(b_prep2[0:DIM, :], bh_d[:, 0:NHYP])
                nc.gpsimd.dma_start(b_prep2[DIM:, :], bh_d[:, NHYP:])

            # rows 0-63: W[i, o*9+t]; rows 64-127: same shifted by one tap so
            # a K=128 conv matmul contracts taps (dy,0) and (dy,1) at once.
            w_conv2 = consts.tile([128, DIM * 9], BF16)

            # ---- main pools ----
            with (
                tc.tile_pool(name="whpool", bufs=1) as whpool,
                tc.tile_pool(name="dram", bufs=1, space="DRAM") as dram,
                tc.tile_pool(name="xin", bufs=3) as xin,
                tc.tile_pool(name="xact", bufs=3) as xactp,
                tc.tile_pool(name="yact", bufs=3) as yactp,
                tc.tile_pool(name="tmp", bufs=6) as tmpp,
                tc.tile_pool(name="outs", bufs=2) as outsp,
                tc.tile_pool(name="ps_s", bufs=4, space="PSUM") as ps_sp,
                tc.tile_pool(name="ps_c", bufs=2, space="PSUM") as ps_cp,
                tc.tile_pool(name="ps_u", bufs=2, space="PSUM") as ps_up,
            ):
                # ---- prologue ----
                wh_sb = whpool.tile([128, n_i * NHYP], BF16, tag="wh")
                if USE_CC:
                    nc.scalar.dma_start(wh_sb[:], wh_d[:])
                else:
                    for q in range(NCHUNK):
                        sl = slice(q * ICH * NHYP, (q + 1) * ICH * NHYP)
                        nc.scalar.dma_start(wh_sb[:, sl], wh_d[:, sl])

                def load_x(img):
                    xT = xin.tile([128, KCH, PIX], BF16, tag="x", name=f"x{img}")
                    nc.sync.dma_start(
                        xT[:].rearrange("p k n -> p (k n)"), x_d[img]
                    )
                    return xT

                xTs = [load_x(0), load_x(1)]

                def make_xact(img):
                    x_act = xactp.tile([128, PAD], BF16, tag="xa", name=f"xa{img}")
                    nc.gpsimd.memset(x_act[:], 0.0)
                    return x_act

                xacts = [make_xact(0), make_xact(1)]

                wh_v = wh_sb[:].rearrange("p (il f) -> p il f", f=NHYP)

                def down(img, xT):
                    """down-proj matmuls -> 2 psum tiles [128, 392]"""
                    ps_ds = [
                        ps_sp.tile([128, NHALF], F32, tag="pss", name=f"psd{img}_{rh}")
                        for rh in range(RH)
                    ]
                    for k in range(KCH):
                        for rh in range(RH):
                            nc.tensor.matmul(
                                ps_ds[rh][:],
                                w_down2[:, k, :],
                                xT[:, k, rh * NHALF : (rh + 1) * NHALF],
                                start=(k == 0),
                                stop=(k == KCH - 1),
                            )
                    return ps_ds

                def gelu1(img, ps_ds, x_act):
                    """quickgelu -> padded interior of x_act (both copies)"""
                    x_act_v = x_act[:].rearrange("d (r c) -> d r c", c=PW)
                    for rh in range(RH):
                        ps_d = ps_ds[rh]
                        t_t = tmpp.tile([128, NHALF], BF16, tag="t")
                        nc.vector.tensor_scalar_add(t_t[:], ps_d[:], b_down2[:])
                        s_t = tmpp.tile([128, NHALF], BF16, tag="s")
                        nc.scalar.activation(
                            s_t[:],
                            ps_d[:],
                            mybir.ActivationFunctionType.Sigmoid,
                            bias=b_down_g2[:],
                            scale=GELU_A,
                        )
                        rows = slice(1 + rh * RROWS, 1 + (rh + 1) * RROWS)
                        nc.vector.tensor_tensor(
                            x_act_v[:DIM, rows, 1 : 1 + W],
                            t_t[:DIM].rearrange("d (r c) -> d r c", c=W),
                            s_t[:DIM].rearrange("d (r c) -> d r c", c=W),
                            mybir.AluOpType.mult,
                        )
                        nc.vector.tensor_tensor(
                            x_act_v[DIM:, rows, 0:W],
                            t_t[DIM:].rearrange("d (r c) -> d r c", c=W),
                            s_t[DIM:].rearrange("d (r c) -> d r c", c=W),
                            mybir.AluOpType.mult,
                        )
                    return x_act_v

                if USE_CC:
                    # hypernet matmuls: psum row il collects W[i0+il, o<32],
                    # row 64+il collects W[i0+il, o>=32] (disjoint rows, one
                    # accumulation group).
                    ps_w = ps_up.tile([128, NHYP], F32, tag="psu", name="hyp")
                    for il in range(n_i):
                        nc.tensor.matmul(
                            ps_w[:],
                            t2[:, 64 - il : 192 - il],
                            wh_v[:, il, :],
                            start=(il == 0),
                            stop=(il == n_i - 1),
                        )
                    # first image's down + gelu1 issued before the collective
                    # so nothing per-image waits behind it in any queue
                    ps_cur = down(0, xTs[0])
                    xact_cur = gelu1(0, ps_cur, xacts[0])
                    blk = whpool.tile([128, NHYP], BF16, tag="blk")
                    nc.vector.tensor_copy(blk[:], ps_w[:])
                    cc_in = dram.tile([2 * ISH, NHYP], BF16)
                    nc.gpsimd.dma_start(cc_in[0:ISH, :], blk[0:ISH, :])
                    nc.gpsimd.dma_start(cc_in[ISH:, :], blk[DIM : DIM + ISH, :])
                    cc_out = dram.tile([2 * DIM, NHYP], BF16)
                    nc.gpsimd.collective_compute(
                        "AllGather",
                        mybir.AluOpType.bypass,
                        replica_groups=[list(range(NCORES))],
                        ins=[cc_in[:].opt()],
                        outs=[cc_out[:].opt()],
                    )
                    # cc_out rows = [(core, half, il)] -> w_conv2[i=8c+il]
                    wtmp = whpool.tile([DIM, DIM * 9], BF16, tag="wtmp")
                    cc_v = cc_out[:].rearrange(
                        "(c hh il) f -> c hh il f", hh=2, il=ISH
                    )
                    for hh in range(2):
                        nc.gpsimd.dma_start(
                            wtmp[:, hh * NHYP : (hh + 1) * NHYP].rearrange(
                                "(c il) f -> c il f", il=ISH
                            ),
                            cc_v[:, hh],
                        )
                    nc.vector.tensor_tensor(
                        w_conv2[:DIM, :], wtmp[:], b_prep[:], mybir.AluOpType.add
                    )
                    # bottom half = top shifted one tap (partition move -> DMA)
                    nc.gpsimd.dma_start(
                        w_conv2[DIM:, : DIM * 9 - 1], w_conv2[:DIM, 1 : DIM * 9]
                    )
                else:
                    # full hypernet: matmuls chase the streaming w_hyper DMA
                    # chunk by chunk (region-level deps); psum/build work in
                    # two 32-row groups (engine partition slices need 32
                    # alignment). Group g's psum rows [32g,32g+32) =
                    # W[i, o<32], rows [64+32g, ..) = W[i, o>=32].
                    for g in range(DIM // IGRP):
                        ps_q = ps_up.tile([128, NHYP], F32, tag="psu", name=f"hyp{g}")
                        for il in range(g * IGRP, (g + 1) * IGRP):
                            nc.tensor.matmul(
                                ps_q[:],
                                t2[:, 64 - il : 192 - il],
                                wh_v[:, il, :],
                                start=(il % IGRP == 0),
                                stop=(il % IGRP == IGRP - 1),
                            )
                        if g == 0:
                            # first image's down+gelu1 fills the PE while the
                            # second half of w_hyper streams in
                            ps_cur = down(0, xTs[0])
                            xact_cur = gelu1(0, ps_cur, xacts[0])
                        rt = slice(g * IGRP, (g + 1) * IGRP)
                        rb = slice(DIM + g * IGRP, DIM + (g + 1) * IGRP)
                        nc.vector.tensor_tensor(
                            w_conv2[rt, :NHYP], ps_q[rt, :], b_prep2[rt, :],
                            mybir.AluOpType.add,
                        )
                        t_b = tmpp.tile([128, NHYP], BF16, tag="t")
                        nc.vector.tensor_tensor(
                            t_b[rb, :], ps_q[rb, :], b_prep2[rb, :],
                            mybir.AluOpType.add,
                        )
                        nc.gpsimd.dma_start(w_conv2[rt, NHYP:], t_b[rb, :])
                        # bottom half rows = top rows shifted by one tap
                        nc.gpsimd.dma_start(
                            w_conv2[rb, : DIM * 9 - 1], w_conv2[rt, 1 : DIM * 9]
                        )
                nc.vector.memset(w_conv2[DIM:, DIM * 9 - 1 :], 0.0)
                w_conv_v = w_conv2[:].rearrange("i (o t) -> i o t", t=9)

                for img in range(B_LOC):
                    # conv: dx=0,1 packed (K=128) + dx=2 (K=64)
                    ps_cs = []
                    for rh in range(RH):
                        ps_c = ps_cp.tile(
                            [DIM, NHALF], F32, tag="psc", name=f"psc{img}_{rh}"
                        )
                        first = True
                        for dy in range(3):
                            src = xact_cur[
                                :, rh * RROWS + dy : rh * RROWS + dy + RROWS, 0:W
                            ]
                            nc.tensor.matmul(
                                ps_c[:],
                                w_conv_v[:, :, dy * 3],
                                src,
                                start=first,
                                stop=False,
                            )
                            first = False
                        for dy in range(3):
                            src = xact_cur[
                                :DIM,
                                rh * RROWS + dy : rh * RROWS + dy + RROWS,
                                2 : 2 + W,
                            ]
                            nc.tensor.matmul(
                                ps_c[:],
                                w_conv_v[:DIM, :, dy * 3 + 2],
                                src,
                                start=False,
                                stop=(dy == 2),
                            )
                        ps_cs.append(ps_c)

                    # pipelined: issue next image's load+down (tensor queue
                    # stays busy while gelu2 below runs on vector/scalar)
                    if img + 1 < B_LOC:
                        if img + 2 < B_LOC:
                            xTs.append(load_x(img + 2))
                            xacts.append(make_xact(img + 2))
                        ps_nxt = down(img + 1, xTs[img + 1])
                    else:
                        ps_nxt = None

                    # gelu2 -> y_act (ones row fuses the up bias)
                    y_act = yactp.tile([DIM + 1, PIX], BF16, tag="ya")
                    nc.gpsimd.memset(y_act[DIM : DIM + 1, :], 1.0)
                    for rh in range(RH):
                        ps_c = ps_cs[rh]
                        t2s = tmpp.tile([DIM, NHALF], BF16, tag="t")
                        nc.vector.tensor_scalar_mul(t2s[:], ps_c[:], scale_sb[:])
                        s2 = tmpp.tile([DIM, NHALF], BF16, tag="s")
                        nc.scalar.activation(
                            s2[:],
                            ps_c[:],
                            mybir.ActivationFunctionType.Sigmoid,
                            bias=0.0,
                            scale=scale_g[:],
                        )
                        nc.vector.tensor_tensor(
                            y_act[:DIM, rh * NHALF : (rh + 1) * NHALF],
                            t2s[:],
                            s2[:],
                            mybir.AluOpType.mult,
                        )

                    # up-proj + bias, transposed: out^T[c,pix] per c-chunk
                    o_sb = outsp.tile([128, KCH, PIX], BF16, tag="o")
                    for kc in range(KCH):
                        for rh in range(RH):
                            ps_u = ps_up.tile([128, NHALF], F32, tag="psu")
                            nc.tensor.matmul(
                                ps_u[:],
                                w_up65[:, kc * 128 : (kc + 1) * 128],
                                y_act[:, rh * NHALF : (rh + 1) * NHALF],
                                start=True,
                                stop=True,
                            )
                            dst = o_sb[:, kc, rh * NHALF : (rh + 1) * NHALF]
                            if (kc * RH + rh) % 2 == 0:
                                nc.scalar.copy(dst, ps_u[:])
                            else:
                                nc.vector.tensor_copy(dst, ps_u[:])
                    nc.scalar.dma_start(
                        out_d[img], o_sb[:].rearrange("p k n -> p (k n)")
                    )

                    if ps_nxt is not None:
                        xact_cur = gelu1(img + 1, ps_nxt, xacts[img + 1])
                        ps_cur = ps_nxt

    nc.compile()
    _CACHE[key] = nc
    return nc


def _pack_hyper_full(w_hyper_bf16):
    """[64e, 36864] -> [128, n_i, 32, 9] packed: rows 0-63 stream the o<32
    block, rows 64-127 the o>=32 block; free layout [il, ol, t]."""
    wh = np.asarray(w_hyper_bf16).reshape(EMB, DIM, DIM, 9)  # [e, o, i, t]
    top = wh[:, :32].transpose(0, 2, 1, 3)  # [e, i, ol, t]
    bot = wh[:, 32:].transpose(0, 2, 1, 3)
    return np.concatenate([top, bot], axis=0)  # [128, i, ol, t]


def _make_in_maps(inputs):
    bf16 = ml_dtypes.bfloat16
    x = np.ascontiguousarray(inputs["x"], dtype=np.float32)
    shared = {
        k: np.ascontiguousarray(inputs[k], np.float32)
        for k in ("w_down", "b_down", "w_up", "b_up", "scale", "layer_emb")
    }
    # bias pre-arranged to [i, (o t)] bf16
    bh = np.asarray(inputs["b_hyper"], np.float32).reshape(DIM, DIM, 9)
    shared["b_hyper"] = np.ascontiguousarray(
        bh.transpose(1, 0, 2).reshape(DIM, DIM * 9)
    ).astype(bf16)

    whb = np.asarray(inputs["w_hyper"], np.float32).astype(bf16)
    packed = _pack_hyper_full(whb)  # [128, i, ol, t]
    if USE_CC:
        wh_packs = [
            np.ascontiguousarray(
                packed[:, c * ISH : (c + 1) * ISH].reshape(128, ISH * NHYP)
            )
            for c in range(NCORES)
        ]
    else:
        full = np.ascontiguousarray(packed.reshape(128, DIM * NHYP))
        wh_packs = [full] * NCORES

    in_maps = []
    for c in range(NCORES):
        xc = x[c * B_LOC : (c + 1) * B_LOC].reshape(B_LOC, PIX, KCH, 128)
        xt = np.ascontiguousarray(xc.transpose(0, 3, 2, 1)).astype(bf16)
        in_maps.append(
            {"x": xt.reshape(B_LOC, 128, KCH * PIX), "w_hyper": wh_packs[c], **shared}
        )
    return in_maps


def _untranspose_out(res):
    outs = []
    for c in range(NCORES):
        o = np.asarray(res.results[c]["out"]).reshape(B_LOC, 128, KCH, PIX)
        o = o.transpose(0, 3, 2, 1).astype(np.float32)  # [img, pix, kc, p]
        outs.append(o.reshape(B_LOC, H, W, C))
    return np.concatenate(outs, axis=0)


def kernel(**inputs) -> np.ndarray:
    nc = build_kernel()
    in_maps = _make_in_maps(inputs)
    res = run_bass_kernel_spmd(nc, in_maps, core_ids=list(range(NCORES)))
    return _untranspose_out(res)


def run_traced(inputs, **kw):
    """For test.py: run with tracing to get HW exec time."""
    nc = build_kernel()
    in_maps = _make_in_maps(inputs)
    return run_bass_kernel_spmd(
        nc, in_maps, core_ids=list(range(NCORES)), trace=True, **kw
    )


# revision 22
# speedup vs baseline: 1.3367x; 1.0647x over previous
"""Trainium2 Bass kernel: Convpass adapter with hypernet-generated 3x3 conv.

Per core (data-parallel over batch, 8 images/core):
  hypernet: conv_w = emb @ w_hyper + b_hyper, via the diag-window matmul
            trick with both o-halves packed on 128 partitions (64 matmuls
            of N=288). The 4.7MB bf16 w_hyper streams in 8 DMA chunks with
            matmuls chasing the chunks; w_conv2 is built in two 32-row
            groups overlapped with the stream.
  down:     xT[128c,4k,784] @ [w_down|w_down] -> psum [128, 392] per half
            (x arrives pre-transposed bf16 from the host; psum rows 0-63 ==
            rows 64-127 so the conv can pack 2 taps)
  gelu1:    quickgelu(x+b) as a single scalar-engine Gelu_apprx_sigmoid
            activation from psum, written twice: rows 0-63 at col+1 (dx=0
            taps), rows 64-127 at col (dx=1 taps) of a padded buffer
  conv:     3x3 as 3 K=128 matmuls (dx=0,1 packed) + 3 K=64 (dx=2)
  gelu2:    quickgelu(scale*y) as one activation per half into y_act
  up:       out^T[128c,392] = w_up65[:,cslice].T @ y_act  (stationary w_up,
            ones-row fused bias); stored transposed bf16, host untransposes.

All small constants (w_down2 dup, w_up65, the hypernet lhsT window tensor,
the rearranged conv bias) are assembled host-side in bf16 and loaded as one
contiguous tensor over the fast hardware DGE ring — the software-DGE cast
path measured 20+us of serialized small packets and gated the first matmul.
The image loop is software-pipelined for the in-order PE queue: tensor
order is conv(i), down(i+1), up(i), with gelu1(i+1) issued right after
down(i+1) so its activations land early in the scalar queue.
"""

import os

import numpy as np
import ml_dtypes

import concourse.bass as bass
import concourse.mybir as mybir
import concourse.tile as tile
from concourse import bacc
from concourse.bass_utils import run_bass_kernel_spmd

# Problem shapes (hardcoded per contract).
B, H, W, C = 64, 28, 28, 512
DIM, EMB = 64, 64
NCORES = 8
B_LOC = B // NCORES            # 8 images per core
PIX = H * W                    # 784 pixels per image
PW = W + 2                     # 30 padded width
PAD = PW * (H + 2)             # 900 padded pixels per image
RH = 2                         # row-halves per image
RROWS = H // RH                # 14 rows per half
NHALF = RROWS * W              # 392 pixels per half-tile
KCH = C // 128                 # 4 contraction chunks of 128 channels
JTOT = DIM * DIM * 9           # 36864 hypernet outputs
NHYP = 32 * 9                  # 288 = free size of packed hypernet matmuls

NCHUNK = 8                     # w_hyper streaming DMA chunks
ICH = DIM // NCHUNK            # 8 i-rows per DMA chunk
IGRP = 32                      # i-rows per compute/build group (32-aligned)

# packed-const column offsets (bf16 [128, CPACK_W])
CP_WDOWN = 0                   # [128, 512]  w_down duplicated, (k m) layout
CP_WUP = 512                   # [65, 512]   w_up with bias row 64
CP_T2 = 1024                   # [128, 192]  hypernet lhsT window tensor
CP_BPREP = 1216                # [128, 288]  conv bias, psum-row layout
CPACK_W = 1504

F32 = mybir.dt.float32
BF16 = mybir.dt.bfloat16
GELU_A = 1.702
# CoreSim doesn't implement Gelu_apprx_sigmoid; substitute Sigmoid for
# structure-only sim runs (numerics then checked on HW via --randup).
ACT_QGELU = (
    mybir.ActivationFunctionType.Sigmoid
    if os.environ.get("KERNEL_DEBUG_SIM_ACT") == "1"
    else mybir.ActivationFunctionType.Gelu_apprx_sigmoid
)

_CACHE = {}


def build_kernel():
    if "nc" in _CACHE:
        return _CACHE["nc"]

    nc = bacc.Bacc("TRN2", target_bir_lowering=False, debug=False)

    x_d = nc.dram_tensor("x", [B_LOC, 128, KCH * PIX], BF16, kind="ExternalInput")
    cpk_d = nc.dram_tensor("cpack", [128, CPACK_W], BF16, kind="ExternalInput")
    cf_d = nc.dram_tensor("cf32", [128, 2], F32, kind="ExternalInput")
    # host-packed hypernet: [128, i, ol, t]; rows 0-63 = o<32, 64-127 = o>=32
    wh_d = nc.dram_tensor("w_hyper", [128, DIM * NHYP], BF16, kind="ExternalInput")
    out_d = nc.dram_tensor("out", [B_LOC, 128, KCH * PIX], BF16, kind="ExternalOutput")

    with tile.TileContext(nc) as tc:
        with tc.tile_pool(name="consts", bufs=1) as consts:
            # ---- constants: 6 contiguous HWDGE loads ----
            w_down2 = consts.tile([128, KCH, 128], BF16)
            nc.sync.dma_start(
                w_down2[:].rearrange("p k m -> p (k m)"),
                cpk_d[:, CP_WDOWN : CP_WDOWN + 512],
            )
            w_up65 = consts.tile([DIM + 1, C], BF16)
            nc.sync.dma_start(w_up65[:], cpk_d[0 : DIM + 1, CP_WUP : CP_WUP + 512])
            t2 = consts.tile([128, 192], BF16)
            nc.sync.dma_start(t2[:], cpk_d[:, CP_T2 : CP_T2 + 192])
            b_prep2 = consts.tile([128, NHYP], BF16)
            nc.sync.dma_start(b_prep2[:], cpk_d[:, CP_BPREP : CP_BPREP + NHYP])
            b_down2 = consts.tile([128, 1], F32)
            nc.sync.dma_start(b_down2[:], cf_d[:, 0:1])
            scale_sb = consts.tile([DIM, 1], F32)
            nc.sync.dma_start(scale_sb[:], cf_d[0:DIM, 1:2])

            # rows 0-63: W[i, o*9+t]; rows 64-127: same shifted by one tap so
            # a K=128 conv matmul contracts taps (dy,0) and (dy,1) at once.
            w_conv2 = consts.tile([128, DIM * 9], BF16)

            # ---- main pools ----
            with (
                tc.tile_pool(name="whpool", bufs=1) as whpool,
                tc.tile_pool(name="xin", bufs=3) as xin,
                tc.tile_pool(name="xact", bufs=3) as xactp,
                tc.tile_pool(name="yact", bufs=3) as yactp,
                tc.tile_pool(name="tmp", bufs=6) as tmpp,
                tc.tile_pool(name="outs", bufs=2) as outsp,
                tc.tile_pool(name="ps_s", bufs=4, space="PSUM") as ps_sp,
                tc.tile_pool(name="ps_c", bufs=2, space="PSUM") as ps_cp,
                tc.tile_pool(name="ps_u", bufs=2, space="PSUM") as ps_up,
            ):
                # ---- prologue ----
                # one tile PER w_hyper chunk: tile-granular dependency
                # tracking would stall the first hypernet matmul until the
                # last chunk landed if this were a single tile
                wh_chunks = []
                for q in range(NCHUNK):
                    cw = ICH * NHYP
                    t = whpool.tile([128, cw], BF16, tag=f"wh{q}")
                    nc.scalar.dma_start(t[:], wh_d[:, q * cw : (q + 1) * cw])
                    wh_chunks.append(t)

                def load_x(img):
                    xT = xin.tile([128, KCH, PIX], BF16, tag="x", name=f"x{img}")
                    nc.sync.dma_start(
                        xT[:].rearrange("p k n -> p (k n)"), x_d[img]
                    )
                    return xT

                xTs = [load_x(0), load_x(1)]

                def make_xact(img):
                    x_act = xactp.tile([128, PAD], BF16, tag="xa", name=f"xa{img}")
                    nc.gpsimd.memset(x_act[:], 0.0)
                    return x_act

                xacts = [make_xact(0), make_xact(1)]

                def wh_slice(il):
                    """rhs [128, 288] for hypernet row il, from its chunk."""
                    t = wh_chunks[il // ICH]
                    j = il % ICH
                    return t[:, j * NHYP : (j + 1) * NHYP]

                def down(img, xT):
                    """down-proj matmuls -> 2 psum tiles [128, 392]"""
                    ps_ds = [
                        ps_sp.tile([128, NHALF], F32, tag="pss", name=f"psd{img}_{rh}")
                        for rh in range(RH)
                    ]
                    for k in range(KCH):
                        for rh in range(RH):
                            nc.tensor.matmul(
                                ps_ds[rh][:],
                                w_down2[:, k, :],
                                xT[:, k, rh * NHALF : (rh + 1) * NHALF],
                                start=(k == 0),
                                stop=(k == KCH - 1),
                            )
                    return ps_ds

                def gelu1(img, ps_ds, x_act):
                    """quickgelu(x+b) = Gelu_apprx_sigmoid(1.0*x + b) straight
                    from psum into the padded interior (both shifted copies)"""
                    x_act_v = x_act[:].rearrange("d (r c) -> d r c", c=PW)
                    for rh in range(RH):
                        ps_d = ps_ds[rh]
                        rows = slice(1 + rh * RROWS, 1 + (rh + 1) * RROWS)
                        nc.scalar.activation(
                            x_act_v[:DIM, rows, 1 : 1 + W],
                            ps_d[:DIM].rearrange("d (r c) -> d r c", c=W),
                            ACT_QGELU,
                            bias=b_down2[:DIM],
                            scale=1.0,
                        )
                        nc.scalar.activation(
                            x_act_v[DIM:, rows, 0:W],
                            ps_d[DIM:].rearrange("d (r c) -> d r c", c=W),
                            ACT_QGELU,
                            bias=b_down2[DIM:],
                            scale=1.0,
                        )
                    return x_act_v

                # full hypernet: matmuls chase the streaming w_hyper DMA
                # chunk by chunk (region-level deps); psum/build work in two
                # 32-row groups (engine partition slices need 32 alignment).
                # Group g's psum rows [32g,32g+32) = W[i, o<32], rows
                # [64+32g, ..) = W[i, o>=32].
                for g in range(DIM // IGRP):
                    ps_q = ps_up.tile([128, NHYP], F32, tag="psu", name=f"hyp{g}")
                    for il in range(g * IGRP, (g + 1) * IGRP):
                        nc.tensor.matmul(
                            ps_q[:],
                            t2[:, 64 - il : 192 - il],
                            wh_slice(il),
                            start=(il % IGRP == 0),
                            stop=(il % IGRP == IGRP - 1),
                        )
                    if g == 0:
                        # first image's down+gelu1 fills the PE while the
                        # second half of w_hyper streams in
                        ps_cur = down(0, xTs[0])
                        xact_cur = gelu1(0, ps_cur, xacts[0])
                    rt = slice(g * IGRP, (g + 1) * IGRP)
                    rb = slice(DIM + g * IGRP, DIM + (g + 1) * IGRP)
                    nc.vector.tensor_tensor(
                        w_conv2[rt, :NHYP], ps_q[rt, :], b_prep2[rt, :],
                        mybir.AluOpType.add,
                    )
                    t_b = tmpp.tile([128, NHYP], BF16, tag="t")
                    nc.vector.tensor_tensor(
                        t_b[rb, :], ps_q[rb, :], b_prep2[rb, :],
                        mybir.AluOpType.add,
                    )
                    nc.gpsimd.dma_start(w_conv2[rt, NHYP:], t_b[rb, :])
                    # bottom half rows = top rows shifted by one tap
                    nc.gpsimd.dma_start(
                        w_conv2[rb, : DIM * 9 - 1], w_conv2[rt, 1 : DIM * 9]
                    )
                nc.vector.memset(w_conv2[DIM:, DIM * 9 - 1 :], 0.0)
                w_conv_v = w_conv2[:].rearrange("i (o t) -> i o t", t=9)

                for img in range(B_LOC):
                    # conv: dx=0,1 packed (K=128) + dx=2 (K=64)
                    ps_cs = []
                    for rh in range(RH):
                        ps_c = ps_cp.tile(
                            [DIM, NHALF], F32, tag="psc", name=f"psc{img}_{rh}"
                        )
                        first = True
                        for dy in range(3):
                            src = xact_cur[
                                :, rh * RROWS + dy : rh * RROWS + dy + RROWS, 0:W
                            ]
                            nc.tensor.matmul(
                                ps_c[:],
                                w_conv_v[:, :, dy * 3],
                                src,
                                start=first,
                                stop=False,
                            )
                            first = False
                        for dy in range(3):
                            src = xact_cur[
                                :DIM,
                                rh * RROWS + dy : rh * RROWS + dy + RROWS,
                                2 : 2 + W,
                            ]
                            nc.tensor.matmul(
                                ps_c[:],
                                w_conv_v[:DIM, :, dy * 3 + 2],
                                src,
                                start=False,
                                stop=(dy == 2),
                            )
                        ps_cs.append(ps_c)

                    # pipelined: issue next image's load+down+gelu1 now —
                    # the gelu1 activations land early in the scalar queue so
                    # conv(img+1) never waits on them, and the down matmuls
                    # keep the PE busy while gelu2 below drains conv psum
                    if img + 1 < B_LOC:
                        if img + 2 < B_LOC:
                            xTs.append(load_x(img + 2))
                            xacts.append(make_xact(img + 2))
                        ps_nxt = down(img + 1, xTs[img + 1])
                        xact_nxt = gelu1(img + 1, ps_nxt, xacts[img + 1])
                    else:
                        ps_nxt = None

                    # gelu2: quickgelu(scale*y) = Gelu_apprx_sigmoid(scale*y)
                    # straight from psum into y_act (ones row fuses up bias)
                    y_act = yactp.tile([DIM + 1, PIX], BF16, tag="ya")
                    nc.vector.memset(y_act[DIM : DIM + 1, :], 1.0)
                    for rh in range(RH):
                        nc.scalar.activation(
                            y_act[:DIM, rh * NHALF : (rh + 1) * NHALF],
                            ps_cs[rh][:],
                            ACT_QGELU,
                            bias=0.0,
                            scale=scale_sb[:],
                        )

                    # up-proj + bias, transposed: out^T[c,pix] per c-chunk;
                    # the output DMA goes out in two halves so the last
                    # image's store starts before its second half is copied
                    o_sb = outsp.tile([128, KCH, PIX], BF16, tag="o")
                    for kc in range(KCH):
                        for rh in range(RH):
                            ps_u = ps_up.tile([128, NHALF], F32, tag="psu")
                            nc.tensor.matmul(
                                ps_u[:],
                                w_up65[:, kc * 128 : (kc + 1) * 128],
                                y_act[:, rh * NHALF : (rh + 1) * NHALF],
                                start=True,
                                stop=True,
                            )
                            dst = o_sb[:, kc, rh * NHALF : (rh + 1) * NHALF]
                            j = kc * RH + rh
                            if j in (1, 3, 5):
                                nc.scalar.copy(dst, ps_u[:])
                            else:
                                nc.vector.tensor_copy(dst, ps_u[:])
                        if kc == 1:
                            nc.scalar.dma_start(
                                out_d[img][:, : 2 * PIX],
                                o_sb[:, 0:2, :].rearrange("p k n -> p (k n)"),
                            )
                    nc.scalar.dma_start(
                        out_d[img][:, 2 * PIX :],
                        o_sb[:, 2:4, :].rearrange("p k n -> p (k n)"),
                    )

                    if ps_nxt is not None:
                        xact_cur = xact_nxt
                        ps_cur = ps_nxt

    nc.compile()
    _CACHE["nc"] = nc
    return nc


def _make_in_maps(inputs):
    bf16 = ml_dtypes.bfloat16
    x = np.ascontiguousarray(inputs["x"], dtype=np.float32)

    # ---- packed bf16 consts ----
    cpk = np.zeros((128, CPACK_W), dtype=bf16)
    wd = np.asarray(inputs["w_down"], np.float32).astype(bf16)
    t = wd.reshape(KCH, 128, DIM).transpose(1, 0, 2)       # [p, k, d]
    cpk[:, CP_WDOWN : CP_WDOWN + 512] = np.concatenate(
        [t, t], axis=2
    ).reshape(128, 512)
    cpk[0:DIM, CP_WUP : CP_WUP + 512] = np.asarray(
        inputs["w_up"], np.float32
    ).astype(bf16)
    cpk[DIM, CP_WUP : CP_WUP + 512] = np.asarray(
        inputs["b_up"], np.float32
    ).astype(bf16)
    emb = np.asarray(inputs["layer_emb"], np.float32).astype(bf16)
    cpk[0:EMB, CP_T2 + 64] = emb
    cpk[EMB:128, CP_T2 + 128] = emb
    bh = np.asarray(inputs["b_hyper"], np.float32).reshape(DIM, DIM, 9)
    b_ot = bh.transpose(1, 0, 2).astype(bf16)              # [i, o, t]
    cpk[0:DIM, CP_BPREP : CP_BPREP + NHYP] = b_ot[:, :32].reshape(DIM, NHYP)
    cpk[DIM:, CP_BPREP : CP_BPREP + NHYP] = b_ot[:, 32:].reshape(DIM, NHYP)

    cf = np.zeros((128, 2), np.float32)
    bd = np.asarray(inputs["b_down"], np.float32)
    cf[0:DIM, 0] = bd
    cf[DIM:, 0] = bd
    cf[0:DIM, 1] = np.asarray(inputs["scale"], np.float32)

    # ---- packed hypernet: [128, i, ol, t]; rows 0-63 = o<32 block ----
    wh = np.asarray(inputs["w_hyper"], np.float32).astype(bf16)
    wh = wh.reshape(EMB, DIM, DIM, 9)                      # [e, o, i, t]
    top = wh[:, :32].transpose(0, 2, 1, 3)                 # [e, i, ol, t]
    bot = wh[:, 32:].transpose(0, 2, 1, 3)
    whp = np.ascontiguousarray(
        np.concatenate([top, bot], axis=0).reshape(128, DIM * NHYP)
    )

    shared = {"cpack": cpk, "cf32": cf, "w_hyper": whp}
    in_maps = []
    for c in range(NCORES):
        xc = x[c * B_LOC : (c + 1) * B_LOC].reshape(B_LOC, PIX, KCH, 128)
        xt = np.ascontiguousarray(xc.transpose(0, 3, 2, 1)).astype(bf16)
        in_maps.append({"x": xt.reshape(B_LOC, 128, KCH * PIX), **shared})
    return in_maps


def _untranspose_out(res):
    outs = []
    for c in range(NCORES):
        o = np.asarray(res.results[c]["out"]).reshape(B_LOC, 128, KCH, PIX)
        o = o.transpose(0, 3, 2, 1).astype(np.float32)  # [img, pix, kc, p]
        outs.append(o.reshape(B_LOC, H, W, C))
    return np.concatenate(outs, axis=0)


def kernel(**inputs) -> np.ndarray:
    nc = build_kernel()
    in_maps = _make_in_maps(inputs)
    res = run_bass_kernel_spmd(nc, in_maps, core_ids=list(range(NCORES)))
    return _untranspose_out(res)


def run_traced(inputs, **kw):
    """For test.py: run with tracing to get HW exec time."""
    nc = build_kernel()
    in_maps = _make_in_maps(inputs)
    return run_bass_kernel_spmd(
        nc, in_maps, core_ids=list(range(NCORES)), trace=True, **kw
    )
